# revision 12
# baseline (speedup 1.0000x reference)
"""Trainium2 Bass kernel for nn_Mlp_62603443306826 (NeuMF + ragged masked-mean MLP).

Sharding: data-parallel over the batch (1024 -> 8 cores x 128 samples), with
samples dealt to cores round-robin in descending user-length order so all
cores share one ragged schedule. Small weights / NCF gathers are replicated.
BatchNorm runs in training mode (batch statistics over all 1024 samples), so
per-core partial sums / sums-of-squares of the fc1 output are combined with a
tiny [128, 8] AllReduce before normalization.

Device pipeline per core:
  1a. user masked mean on the PE: per sample, mask-column (x 1/len, fp32r)
      as the stationary operand against the sample's [len, 768] feature rows
      (length quantized to {64,128}+{0,72} so every DMA stripes all 16 DMA
      engines); psum row -> stage tile -> small gather-DMA on the ACT ring
      (a separate ring avoids head-of-line blocking behind feature loads).
  1b. hashtag masked mean on the Vector engine: fused multiply-accumulate
      per position over [128, CH, 768] chunks, rows sorted by hashtag length
      and clipped to {64,128} rows.
  2. NeuMF tower (16->32->16->8 MLP, transposed activations) on PE + ACT.
  3. PE transposes user/hashtag embeddings into x.T chunks (the hashtag
     transpose multiplies by a permutation matrix, restoring canonical row
     order); fc1 as 4x13 accumulating fp32 matmuls (hidden on partitions).
  4. BN stats -> AllReduce -> fused scale/shift+relu; fc3 matmul; sigmoid.

fc1_b is intentionally not applied: training-mode BatchNorm subtracts the
batch mean, which cancels any constant bias added after fc1 exactly.
fp32r (TF32-class) is used only for the user masked-sum matmuls
(~1.5e-4 relative); everything else is fp32.
"""

import os
import sys

for _p in ("/opt/trn_rl_repo", "/root/.axon_site/_ro/trn_rl_repo"):
    if os.path.isdir(_p) and _p not in sys.path:
        sys.path.append(_p)

import numpy as np

import concourse.bacc as bacc
import concourse.tile as tile
import concourse.mybir as mybir
from concourse.bass_utils import run_bass_kernel_spmd

N_CORES = 8
B = 1024
S = B // N_CORES  # 128 samples per core
LU, LH, D = 200, 50, 768
HID = 512
EPS = 1e-5
F32 = mybir.dt.float32
F32R = mybir.dt.float32r

HCHUNK = 5   # hashtag positions per DMA chunk (10 chunks)

_nc_cache = {}
last_result = None  # BassKernelResults of the most recent run (for test harness)


def _build_program(k1s, k2s, k_h):
    nc = bacc.Bacc("TRN2", target_bir_lowering=False, debug=False,
                   num_devices=N_CORES)

    def din(name, shape, dt=F32):
        return nc.dram_tensor(name, list(shape), dt, kind="ExternalInput").ap()

    uf = din("uf", (S, LU, D), F32R)
    umT = din("umT", (LU, S), F32R)
    hf = din("hf", (S, LH, D))
    hm = din("hm", (S, LH))
    h0T = din("h0T", (16, S))
    umfT = din("umfT", (8, S))
    imfT = din("imfT", (8, S))
    w0 = din("w0", (16, 32))
    b0 = din("b0", (32, 1))
    w1 = din("w1", (32, 16))
    b1 = din("b1", (16, 1))
    w2 = din("w2", (16, 8))
    b2 = din("b2", (8, 1))
    fc1w = din("fc1w", (1552, HID))
    gamma4 = din("gamma4", (S, 4))
    beta4 = din("beta4", (S, 4))
    fc3w4 = din("fc3w4", (S, 4))
    fc3b = din("fc3b", (S, 1))
    ident = din("ident", (S, S))
    ph = din("ph", (S, S))  # per-core hashtag un-permutation matrix
    out = nc.dram_tensor("out", [S, 1], F32, kind="ExternalOutput").ap()

    MUL = mybir.AluOpType.mult
    ADD = mybir.AluOpType.add
    SUB = mybir.AluOpType.subtract
    AF = mybir.ActivationFunctionType

    with tile.TileContext(nc) as tc:
        with (
            tc.tile_pool(name="consts", bufs=1) as pc,
            tc.tile_pool(name="fuser", bufs=6) as pfu,
            tc.tile_pool(name="fhash", bufs=3) as pfh,
            tc.tile_pool(name="stage", bufs=8) as pst,
            tc.tile_pool(name="acc", bufs=1) as pacc,
            tc.tile_pool(name="work", bufs=1) as pw,
            tc.tile_pool(name="scratch", bufs=2) as psc,
            tc.tile_pool(name="dram", bufs=1, space="DRAM") as pdram,
        ):
            # masks first (the user loop needs them immediately)
            umT0_sb = pc.tile([128, S], F32R)
            umT1_sb = pc.tile([LU - 128, S], F32R)
            hm_sb = pc.tile([S, LH], F32)
            nc.sync.dma_start(umT0_sb[:], umT[0:128, :])
            nc.sync.dma_start(umT1_sb[:], umT[128:LU, :])
            nc.sync.dma_start(hm_sb[:], hm[:])

            # NCF tower consts on the ACT ring (keeps the SP ring for bulk)
            h0T_sb = pc.tile([16, S], F32)
            umfT_sb = pc.tile([8, S], F32)
            imfT_sb = pc.tile([8, S], F32)
            w0_sb = pc.tile([16, 32], F32)
            b0_sb = pc.tile([32, 1], F32)
            w1_sb = pc.tile([32, 16], F32)
            b1_sb = pc.tile([16, 1], F32)
            w2_sb = pc.tile([16, 8], F32)
            b2_sb = pc.tile([8, 1], F32)
            for t, src in (
                (h0T_sb, h0T), (umfT_sb, umfT), (imfT_sb, imfT),
                (w0_sb, w0), (b0_sb, b0), (w1_sb, w1), (b1_sb, b1),
                (w2_sb, w2), (b2_sb, b2),
            ):
                nc.scalar.dma_start(t[:], src[:])

            x_u = pw.tile([S, D], F32)     # user embeddings, canonical order
            acc_h = pacc.tile([S, D], F32)  # hashtag embeddings, hashtag order

            with tc.tile_pool(name="ps_ncf", bufs=2, space="PSUM") as ps_ncf:
                # ---- NeuMF tower (PE/ACT, runs during the masked-mean) ----
                ncfT = pw.tile([16, S], F32)
                p0 = ps_ncf.tile([32, S], F32, tag="ncf")
                nc.tensor.matmul(p0[:], w0_sb[:], h0T_sb[:], start=True, stop=True)
                h1T = pw.tile([32, S], F32)
                nc.scalar.activation(h1T[:], p0[:], AF.Relu, bias=b0_sb[:, 0:1])
                p1 = ps_ncf.tile([16, S], F32, tag="ncf")
                nc.tensor.matmul(p1[:], w1_sb[:], h1T[:], start=True, stop=True)
                h2T = pw.tile([16, S], F32)
                nc.scalar.activation(h2T[:], p1[:], AF.Relu, bias=b1_sb[:, 0:1])
                p2 = ps_ncf.tile([8, S], F32, tag="ncf")
                nc.tensor.matmul(p2[:], w2_sb[:], h2T[:], start=True, stop=True)
                nc.scalar.activation(ncfT[0:8, :], p2[:], AF.Relu, bias=b2_sb[:, 0:1])
                mfT = pw.tile([8, S], F32)
                nc.vector.tensor_tensor(mfT[:], umfT_sb[:], imfT_sb[:], op=MUL)
                nc.scalar.dma_start(ncfT[8:16, :], mfT[:])

                with tc.tile_pool(name="ps_u", bufs=3, space="PSUM") as ps_u:
                    # ---- masked means: user on PE (per sample), hashtag on
                    # DVE (chunked MAC), hashtag chunks interleaved into the
                    # user stream so the SP ring feeds both engines steadily.
                    n_hchunks = (LH + HCHUNK - 1) // HCHUNK

                    def hashtag_chunk(j):
                        l0 = j * HCHUNK
                        k = k_h[j]
                        if k == 0:
                            return
                        fh = pfh.tile([S, HCHUNK, D], F32)
                        nc.sync.dma_start(fh[0:k, :, :], hf[0:k, l0 : l0 + HCHUNK, :])
                        for li in range(HCHUNK):
                            l = l0 + li
                            if l == 0:
                                nc.vector.tensor_scalar_mul(
                                    acc_h[:], fh[:, li, :], hm_sb[:, l : l + 1])
                            else:
                                nc.vector.scalar_tensor_tensor(
                                    acc_h[0:k], fh[0:k, li, :], hm_sb[0:k, l : l + 1],
                                    acc_h[0:k], op0=MUL, op1=ADD)

                    for i in range(S):
                        if i % 13 == 6 and i // 13 < n_hchunks:
                            hashtag_chunk(i // 13)
                        k1, k2 = k1s[i], k2s[i]
                        ft = pfu.tile([128, D], F32R, tag="ftu")
                        nc.sync.dma_start(ft[0:k1, :], uf[i, 0:k1, :])
                        if k2:
                            ft2 = pfu.tile([LU - 128, D], F32R, tag="ftu2")
                            nc.sync.dma_start(ft2[0:k2, :], uf[i, 128 : 128 + k2, :])
                        pt = ps_u.tile([1, D], F32, tag="ptu")
                        mk1 = umT0_sb[0:k1, i : i + 1]
                        nc.tensor.matmul(pt[0:1, 0:512], mk1, ft[0:k1, 0:512],
                                         start=True, stop=(k2 == 0))
                        nc.tensor.matmul(pt[0:1, 512:768], mk1, ft[0:k1, 512:768],
                                         start=True, stop=(k2 == 0))
                        if k2:
                            mk2 = umT1_sb[0:k2, i : i + 1]
                            nc.tensor.matmul(pt[0:1, 0:512], mk2, ft2[0:k2, 0:512],
                                             start=False, stop=True)
                            nc.tensor.matmul(pt[0:1, 512:768], mk2, ft2[0:k2, 512:768],
                                             start=False, stop=True)
                        stage = pst.tile([1, D], F32, tag="stg")
                        nc.scalar.activation(stage[:], pt[:], AF.Copy)
                        # gather into canonical rows via the ACT HWDGE ring
                        nc.scalar.dma_start(x_u[i : i + 1, :], stage[:])

                    # late weight loads on the SP ring: they land as the
                    # masked-mean drains, right before fc1 needs them
                    fc1w_sb = []
                    for c in range(13):
                        kk = 128 if c < 12 else 16
                        t = pc.tile([kk, HID], F32, tag=f"fc1w{c}")
                        nc.sync.dma_start(t[:], fc1w[c * 128 : c * 128 + kk, :])
                        fc1w_sb.append(t)
                    gamma_sb = pc.tile([S, 4], F32)
                    beta_sb = pc.tile([S, 4], F32)
                    fc3w_sb = pc.tile([S, 4], F32)
                    fc3b_sb = pc.tile([S, 1], F32)
                    ident_sb = pc.tile([S, S], F32)
                    ph_sb = pc.tile([S, S], F32)
                    for t, src in (
                        (gamma_sb, gamma4), (beta_sb, beta4), (fc3w_sb, fc3w4),
                        (fc3b_sb, fc3b), (ident_sb, ident), (ph_sb, ph),
                    ):
                        nc.sync.dma_start(t[:], src[:])

            with (
                tc.tile_pool(name="ps_t", bufs=2, space="PSUM") as ps_t,
                tc.tile_pool(name="ps_mm", bufs=2, space="PSUM") as ps_mm,
                tc.tile_pool(name="ps_y", bufs=1, space="PSUM") as ps_y,
            ):
                # ---- transpose embeddings into x.T chunks ------------------
                xT = pw.tile([S, 12 * S], F32)
                for c in range(12):
                    src = x_u if c < 6 else acc_h
                    off = (c % 6) * S
                    pt = ps_t.tile([S, S], F32, tag="tr")
                    if c < 6:
                        nc.tensor.matmul(pt[:], src[:, off : off + S], ident_sb[:],
                                         is_transpose=True)
                    else:
                        nc.tensor.matmul(pt[:], src[:, off : off + S], ph_sb[:],
                                         start=True, stop=True)
                    nc.vector.tensor_copy(xT[:, c * S : (c + 1) * S], pt[:])

                # ---- fc1 (output transposed: hidden on partitions) ---------
                x1 = pw.tile([S, HID], F32)
                stats = pw.tile([S, 8], F32)
                for m in range(4):
                    pm = ps_mm.tile([S, S], F32, tag="fc1")
                    for c in range(13):
                        rhs = xT[:, c * S : (c + 1) * S] if c < 12 else ncfT[:]
                        nc.tensor.matmul(
                            pm[:], fc1w_sb[c][:, m * 128 : (m + 1) * 128], rhs,
                            start=(c == 0), stop=(c == 12))
                    nc.vector.tensor_copy(x1[:, m * 128 : (m + 1) * 128], pm[:])
                    nc.vector.tensor_reduce(
                        stats[:, m : m + 1], x1[:, m * 128 : (m + 1) * 128],
                        axis=mybir.AxisListType.X, op=ADD)
                    sq_scr = psc.tile([S, S], F32, tag="sq")
                    nc.scalar.activation(
                        sq_scr[:], x1[:, m * 128 : (m + 1) * 128], AF.Square,
                        accum_out=stats[:, 4 + m : 5 + m])

                # ---- AllReduce batch stats over the 8 cores ----------------
                cc_in = pdram.tile([S, 8], F32)
                cc_out = pdram.tile([S, 8], F32)
                nc.sync.dma_start(cc_in[:], stats[:])
                nc.gpsimd.collective_compute(
                    "AllReduce", ADD,
                    replica_groups=[list(range(N_CORES))],
                    ins=[cc_in.opt()], outs=[cc_out.opt()])
                red = pw.tile([S, 8], F32)
                nc.sync.dma_start(red[:], cc_out[:])

                # ---- BN coefficients: A = gamma*rsqrt(var+eps), B = beta-mu*A
                mm8 = pw.tile([S, 8], F32)
                nc.vector.tensor_scalar_mul(mm8[:], red[:], 1.0 / B)
                var4 = pw.tile([S, 4], F32)
                nc.vector.tensor_tensor(var4[:], mm8[:, 0:4], mm8[:, 0:4], op=MUL)
                nc.vector.tensor_tensor(var4[:], mm8[:, 4:8], var4[:], op=SUB)
                nc.vector.tensor_scalar_add(var4[:], var4[:], EPS)
                std4 = pw.tile([S, 4], F32)
                nc.scalar.activation(std4[:], var4[:], AF.Sqrt, bias=0.0)
                ab = pw.tile([S, 8], F32)
                nc.vector.reciprocal(ab[:, 0:4], std4[:])
                nc.vector.tensor_tensor(ab[:, 0:4], gamma_sb[:], ab[:, 0:4], op=MUL)
                nc.vector.tensor_tensor(ab[:, 4:8], mm8[:, 0:4], ab[:, 0:4], op=MUL)
                nc.vector.tensor_tensor(ab[:, 4:8], beta_sb[:], ab[:, 4:8], op=SUB)

                # ---- BN + relu fused, then fc3 + sigmoid -------------------
                rT = pw.tile([S, HID], F32)
                for m in range(4):
                    nc.scalar.activation(
                        rT[:, m * 128 : (m + 1) * 128],
                        x1[:, m * 128 : (m + 1) * 128], AF.Relu,
                        bias=ab[:, 4 + m : 5 + m], scale=ab[:, m : m + 1])
                py = ps_y.tile([S, 1], F32)
                for m in range(4):
                    nc.tensor.matmul(py[:], rT[:, m * 128 : (m + 1) * 128],
                                     fc3w_sb[:, m : m + 1],
                                     start=(m == 0), stop=(m == 3))
                out_sb = pw.tile([S, 1], F32)
                nc.scalar.activation(out_sb[:], py[:], AF.Sigmoid, bias=fc3b_sb[:, 0:1])
                nc.sync.dma_start(out[:], out_sb[:])

    nc.compile()
    return nc


def kernel(**inputs) -> np.ndarray:
    global last_result
    uf = np.asarray(inputs["user_features"], np.float32)
    hf = np.asarray(inputs["hashtag_features"], np.float32)
    ul = np.asarray(inputs["user_lens"]).astype(np.int64)
    hl = np.asarray(inputs["hashtag_lens"]).astype(np.int64)
    users = np.asarray(inputs["users"])
    items = np.asarray(inputs["items"])

    um = ((np.arange(LU)[None, :] < ul[:, None]) / ul[:, None]).astype(np.float32)
    hm = ((np.arange(LH)[None, :] < hl[:, None]) / hl[:, None]).astype(np.float32)

    h0 = np.concatenate(
        [np.asarray(inputs["u_mlp"])[users], np.asarray(inputs["i_mlp"])[items]], axis=1
    ).astype(np.float32)  # [B, 16]
    umf = np.asarray(inputs["u_mf"])[users].astype(np.float32)  # [B, 8]
    imf = np.asarray(inputs["i_mf"])[items].astype(np.float32)

    # Deal samples to cores round-robin in descending user-length order:
    # every core then shares one ragged schedule (max over cores, baked in).
    uord = np.argsort(-ul, kind="stable")
    samples = [uord[np.arange(S) * N_CORES + c] for c in range(N_CORES)]
    hperms = [np.argsort(-hl[s], kind="stable") for s in samples]

    # user per-sample K schedule: first chunk in {64, 128}, tail in {0, 72}
    lens_mat = np.stack([ul[s] for s in samples])  # [cores, S]
    lmax = lens_mat.max(axis=0)
    k1s = tuple(64 if v <= 64 else 128 for v in lmax)
    k2s = tuple(0 if v <= 128 else LU - 128 for v in lmax)

    # hashtag chunk schedule, rows sorted by hashtag length, k in {64, 128}
    hl_sorted = np.stack([hl[s][p] for s, p in zip(samples, hperms)])
    k_h = []
    for l0 in range(0, LH, HCHUNK):
        k = int((hl_sorted > l0).sum(axis=1).max())
        k_h.append(0 if k == 0 else (64 if k <= 64 else 128))
    k_h = tuple(k_h)

    C = np.ascontiguousarray
    rep = {
        "w0": C(np.asarray(inputs["mlp_w0"], np.float32)),
        "b0": C(np.asarray(inputs["mlp_b0"], np.float32).reshape(32, 1)),
        "w1": C(np.asarray(inputs["mlp_w1"], np.float32)),
        "b1": C(np.asarray(inputs["mlp_b1"], np.float32).reshape(16, 1)),
        "w2": C(np.asarray(inputs["mlp_w2"], np.float32)),
        "b2": C(np.asarray(inputs["mlp_b2"], np.float32).reshape(8, 1)),
        "fc1w": C(np.asarray(inputs["fc1_w"], np.float32)),
        "gamma4": C(np.asarray(inputs["bn_gamma"], np.float32).reshape(4, 128).T),
        "beta4": C(np.asarray(inputs["bn_beta"], np.float32).reshape(4, 128).T),
        "fc3w4": C(np.asarray(inputs["fc3_w"], np.float32).reshape(4, 128).T),
        "fc3b": np.full((S, 1), np.float32(np.asarray(inputs["fc3_b"]).reshape(-1)[0])),
        "ident": np.eye(S, dtype=np.float32),
    }

    in_maps = []
    for c in range(N_CORES):
        s = samples[c]
        hp = hperms[c]
        P = np.zeros((S, S), np.float32)
        P[np.arange(S), hp] = 1.0  # row r (hashtag order) -> canonical col hp[r]
        m = {
            "uf": C(uf[s]), "umT": C(um[s].T),
            "hf": C(hf[s][hp]), "hm": C(hm[s][hp]),
            "h0T": C(h0[s].T), "umfT": C(umf[s].T), "imfT": C(imf[s].T),
            "ph": P,
        }
        m.update(rep)
        in_maps.append(m)

    key = (k1s, k2s, k_h)
    if key not in _nc_cache:
        _nc_cache.clear()
        _nc_cache[key] = _build_program(k1s, k2s, k_h)
    res = run_bass_kernel_spmd(_nc_cache[key], in_maps, core_ids=list(range(N_CORES)))
    last_result = res

    out = np.empty((B, 1), np.float32)
    for c in range(N_CORES):
        out[samples[c]] = res.results[c]["out"]
    return out
